# revision 15
# baseline (speedup 1.0000x reference)
"""CRFSmooth2D Trainium2 kernel (8-core data parallel over batch).

Per sample b (one NeuronCore each):
  cm[k,c]   = (sum_hw L[k]*I[c]/HW) / (mean(L[k]) + 1e-5)
  diff[k]   = sum_c (I[c]-cm[k,c])^2 = x2sum + sum_c a_c*I[c] + m2,  a_c=-2cm_c
  w[k]      = exp(-4*diff^2)
  out[k]    = conv3x3(L[k]*w[k]) / (conv3x3(w[k]) + 1e-6)
conv3x3 is separable: row pass [A,1,A] (free-dim shifted STT) then column
pass (PE tridiagonal matmul per 128-row block + rank-1 boundary fixups).
"""

import os
import sys

import numpy as np

sys.path.insert(0, "/opt/trn_rl_repo")

import concourse.bass as bass
import concourse.tile as tile
from concourse import bacc, mybir
from concourse.bass_utils import run_bass_kernel_spmd

B, K, C, H, W = 8, 16, 3, 512, 512
HW = H * W
NBLK = H // 128
A = float(np.exp(-2.0))  # 3x3 gaussian off-center weight (sigma=0.5)
F32 = mybir.dt.float32
BF16 = mybir.dt.bfloat16

# Engine/dtype tuning flags
USE_DERF = bool(int(os.environ.get("CRF_USE_DERF", "1")))  # Derivative_Erf fused weight
USE_BF16_P1 = bool(int(os.environ.get("CRF_BF16_P1", "1")))  # bf16 phase-1 stats
USE_BF16_W = bool(int(os.environ.get("CRF_BF16_W", "1")))  # bf16 w / l*w tiles
USE_BF16_RC = bool(int(os.environ.get("CRF_BF16_RC", "0")))  # bf16 rowconv out + tmat
SQ = 514  # padded row length (zero col at 0 and 513)
KMAX = int(os.environ.get("CRF_KMAX", str(K)))  # debug: emit only first KMAX planes
STAGE = int(os.environ.get("CRF_STAGE", "4"))  # debug: 1 stats, 2 weights, 3 rowconv, 4 full

if USE_DERF:
    # w~ = (2/sqrt(pi)) * exp(-4 diff^2) via Derivative_Erf(2*diff); the
    # constant cancels in num/den if eps is scaled by the same factor.
    EPS_DEN = 1e-6 * 2.0 / float(np.sqrt(np.pi))
else:
    EPS_DEN = 1e-6

WDT = BF16 if USE_BF16_W else F32
RDT = BF16 if USE_BF16_RC else F32


def _consts():
    t = np.zeros((128, 128), dtype=np.float32)
    for i in range(128):
        t[i, i] = 1.0
        if i > 0:
            t[i, i - 1] = A
        if i < 127:
            t[i, i + 1] = A
    # fixup lhsT blocks (full 128-row coverage keeps PSUM groups simple):
    # cols 0:128 -> only out row 127 gets A*rhs[0]
    # cols 128:256 -> only out row 0 gets A*rhs[127]
    e = np.zeros((128, 256), dtype=np.float32)
    e[0, 127] = A
    e[127, 128] = A
    if USE_BF16_RC:
        import ml_dtypes

        t = t.astype(ml_dtypes.bfloat16)
        e = e.astype(ml_dtypes.bfloat16)
    return t, e


def _build():
    nc = bacc.Bacc("TRN2", target_bir_lowering=False, debug=False)

    lab_d = nc.dram_tensor("labels", [K, H, W], F32, kind="ExternalInput").ap()
    in_d = nc.dram_tensor("inputs", [C, H, W], F32, kind="ExternalInput").ap()
    tm_d = nc.dram_tensor("tmat", [128, 128], RDT, kind="ExternalInput").ap()
    ev_d = nc.dram_tensor("evec", [128, 256], RDT, kind="ExternalInput").ap()
    out_d = nc.dram_tensor("out", [K, H, W], F32, kind="ExternalOutput").ap()

    lab_r = lab_d.rearrange("k (blk p) w -> k p blk w", p=128)
    in_r = in_d.rearrange("c (blk p) w -> p c blk w", p=128)
    out_r = out_d.rearrange("k (blk p) w -> k p blk w", p=128)

    with tile.TileContext(nc) as tc:
        _emit(tc, lab_r, in_r, tm_d, ev_d, out_r)
    nc.compile()
    return nc


def _emit(tc, lab_r, in_r, tm_d, ev_d, out_r):
    nc = tc.nc
    ex = tc.enter_context if hasattr(tc, "enter_context") else None

    import contextlib

    ctx = contextlib.ExitStack()
    pool = lambda name, bufs, space="SBUF": ctx.enter_context(
        tc.tile_pool(name=name, bufs=bufs, space=space)
    )

    consts = pool("consts", 1)
    ipool = pool("ipool", 1)
    lab = pool("lab", 3)
    scr = pool("scr", 2)
    sml = pool("sml", 2)
    tt = pool("tt", 3)
    wp = pool("wp", 2)
    lwp = pool("lwp", 2)
    wcp = pool("wcp", 2)
    lwcp = pool("lwcp", 2)
    dsp = pool("dsp", 2)
    outp = pool("outp", 2)
    ps_s = pool("ps_s", 1, "PSUM")
    ps_c = pool("ps_c", 3, "PSUM")

    # ---- constants / resident inputs ----
    tmat = consts.tile([128, 128], RDT)
    nc.sync.dma_start(tmat[:], tm_d[:])
    evec = consts.tile([128, 256], RDT)
    nc.sync.dma_start(evec[:], ev_d[:])
    ones = consts.tile([128, 128], F32)
    nc.vector.memset(ones[:], 1.0)

    I = consts.tile([128, C, NBLK, 512], F32)
    nc.sync.dma_start(I[:], in_r[:])
    if USE_BF16_P1:
        Ib = consts.tile([128, C, NBLK, 512], BF16)
        nc.scalar.copy(Ib[:], I[:])

    # x2sum = sum_c I_c^2
    xs = consts.tile([128, NBLK, 512], F32)
    nc.scalar.square(xs[:], I[:, 0])
    sq1 = tt.tile([128, NBLK, 512], F32, tag="t")
    nc.scalar.square(sq1[:], I[:, 1])
    nc.vector.tensor_add(xs[:], xs[:], sq1[:])
    sq2 = tt.tile([128, NBLK, 512], F32, tag="t")
    nc.scalar.square(sq2[:], I[:, 2])
    nc.vector.tensor_add(xs[:], xs[:], sq2[:])

    mul, add = mybir.AluOpType.mult, mybir.AluOpType.add

    for k in range(KMAX):
        # ---- load plane ----
        L = lab.tile([128, NBLK, 512], F32, tag="lab")
        nc.sync.dma_start(L[:], lab_r[k])

        # ---- phase 1: stats ----
        S = sml.tile([128, 4], F32, tag="S")
        Lb = scr.tile([128, NBLK, 512], BF16, tag="lb")
        # bf16 copy of L; accumulate raw label sum on the way
        nc.scalar.activation(
            Lb[:], L[:], mybir.ActivationFunctionType.Copy, accum_out=S[:, 3:4]
        )
        if USE_BF16_P1:
            p1a, p1b = Ib, Lb
        else:
            p1a, p1b = I, L
        for c, eng in ((0, nc.vector), (1, nc.vector), (2, nc.vector)):
            po = scr.tile([128, NBLK, 512], BF16, tag="scr2")
            eng.scalar_tensor_tensor(
                po[:], p1a[:, c] if USE_BF16_P1 else I[:, c],
                1.0 / HW, p1b[:], mul, mul, accum_out=S[:, c : c + 1],
            )

        pss = ps_s.tile([128, 4], F32, tag="pss")
        nc.tensor.matmul(pss[:], ones[:], S[:], start=True, stop=True)
        sums = sml.tile([128, 4], F32, tag="sums")
        nc.scalar.copy(sums[:], pss[:])

        dm = sml.tile([128, 1], F32, tag="dm")
        nc.vector.tensor_scalar(dm[:], sums[:, 3:4], 1.0 / HW, 1e-5, mul, add)
        rden = sml.tile([128, 1], F32, tag="rden")
        nc.vector.reciprocal(rden[:], dm[:])
        cm = sml.tile([128, 3], F32, tag="cm")
        nc.vector.tensor_scalar(cm[:], sums[:, 0:3], rden[:, 0:1], None, mul)
        aa = sml.tile([128, 3], F32, tag="aa")
        nc.vector.tensor_scalar(aa[:], cm[:], -2.0, None, mul)
        # m2s = s * sum_c cm_c^2  (s=2 for DerivErf bias, 1 for Square bias)
        cmsq = sml.tile([128, 3], F32, tag="cmsq")
        m2s = sml.tile([128, 1], F32, tag="m2s")
        nc.vector.scalar_tensor_tensor(
            cmsq[:], cm[:], 2.0 if USE_DERF else 1.0, cm[:], mul, mul,
            accum_out=m2s[:],
        )

        if STAGE < 2:
            odbg = outp.tile([128, NBLK, 512], F32, tag="ot")
            nc.vector.tensor_scalar(odbg[:, 0, 0:4], S[:], 1.0, None, mul)
            nc.sync.dma_start(out_r[k], odbg[:])
            continue
        # ---- phase 2: diff chain (fp32) ----
        t1 = tt.tile([128, NBLK, 512], F32, tag="t")
        nc.vector.scalar_tensor_tensor(t1[:], I[:, 1], aa[:, 1:2], xs[:], mul, add)
        t2 = tt.tile([128, NBLK, 512], F32, tag="t")
        nc.vector.scalar_tensor_tensor(t2[:], I[:, 2], aa[:, 2:3], t1[:], mul, add)
        t3 = tt.tile([128, NBLK, 512], F32, tag="t")
        nc.vector.scalar_tensor_tensor(t3[:], I[:, 0], aa[:, 0:1], t2[:], mul, add)

        # ---- weights ----
        w = wp.tile([128, NBLK, SQ], WDT, tag="w")
        nc.gpsimd.memset(w[:, :, 0:1], 0.0)
        nc.gpsimd.memset(w[:, :, 513:514], 0.0)
        wv = w[:, :, 1:513]
        if USE_DERF:
            # (2/sqrt(pi)) * exp(-(2*diff+2*m2... wait bias folds m2)
            nc.scalar.activation(
                wv, t3[:], mybir.ActivationFunctionType.Derivative_Erf,
                bias=m2s[:, 0:1], scale=2.0,
            )
        else:
            d2 = tt.tile([128, NBLK, 512], F32, tag="t")
            nc.scalar.activation(
                d2[:], t3[:], mybir.ActivationFunctionType.Square,
                bias=m2s[:, 0:1], scale=1.0,
            )
            nc.scalar.activation(
                wv, d2[:], mybir.ActivationFunctionType.Exp, scale=-4.0
            )

        lw = lwp.tile([128, NBLK, SQ], WDT, tag="lw")
        nc.gpsimd.memset(lw[:, :, 0:1], 0.0)
        nc.gpsimd.memset(lw[:, :, 513:514], 0.0)
        lsrc = Lb if USE_BF16_W else L
        nc.gpsimd.tensor_mul(lw[:, :, 1:513], lsrc[:], wv)

        if STAGE < 3:
            odbg = outp.tile([128, NBLK, 512], F32, tag="ot")
            nc.vector.tensor_copy(odbg[:], t3[:])
            nc.sync.dma_start(out_r[k], odbg[:])
            continue
        # ---- row conv (W axis) ----
        wc = wcp.tile([128, NBLK, 512], RDT, tag="wc")
        nc.vector.scalar_tensor_tensor(wc[:], w[:, :, 2:514], A, wv, mul, add)
        nc.vector.scalar_tensor_tensor(wc[:], w[:, :, 0:512], A, wc[:], mul, add)
        lwc = lwcp.tile([128, NBLK, 512], RDT, tag="lwc")
        nc.gpsimd.tensor_scalar(lwc[:], lw[:, :, 2:514], A, None, mul)
        nc.gpsimd.tensor_add(lwc[:], lwc[:], lw[:, :, 1:513])
        lt = lwcp.tile([128, NBLK, 512], WDT, tag="lt")
        nc.gpsimd.tensor_scalar(lt[:], lw[:, :, 0:512], A, None, mul)
        nc.gpsimd.tensor_add(lwc[:], lwc[:], lt[:])

        if STAGE < 4:
            odbg = outp.tile([128, NBLK, 512], F32, tag="ot")
            nc.vector.tensor_copy(odbg[:], wc[:])
            nc.sync.dma_start(out_r[k], odbg[:])
            continue
        # ---- column conv (H axis) via PE + divide ----
        ot = outp.tile([128, NBLK, 512], F32, tag="ot")
        for i in range(NBLK):
            for src, ps_tag in ((wc, "pd"), (lwc, "pn")):
                ps = ps_c.tile([128, 512], F32, tag=ps_tag)
                nc.tensor.matmul(ps[:], tmat[:], src[:, i, :], start=True,
                                 stop=(i == 0 and NBLK == 1))
                if i < NBLK - 1:
                    nc.tensor.matmul(
                        ps[:], evec[:, 0:128], src[:, i + 1, :],
                        start=False, stop=(i == 0),
                    )
                if i > 0:
                    nc.tensor.matmul(
                        ps[:], evec[:, 128:256], src[:, i - 1, :],
                        start=False, stop=True,
                    )
                if src is wc:
                    psd = ps
                else:
                    psn = ps
            dens = dsp.tile([128, 512], F32, tag="ds")
            nc.scalar.activation(
                dens[:], psd[:], mybir.ActivationFunctionType.Copy, bias=EPS_DEN
            )
            r = dsp.tile([128, 512], F32, tag="r")
            nc.vector.reciprocal(r[:], dens[:])
            nc.vector.tensor_mul(ot[:, i, :], psn[:], r[:])

        nc.sync.dma_start(out_r[k], ot[:])

    ctx.close()


_NC_CACHE = {}


def kernel(labels: np.ndarray, inputs: np.ndarray) -> np.ndarray:
    assert labels.shape == (B, K, H, W) and inputs.shape == (B, C, H, W)
    key = "nc"
    if key not in _NC_CACHE:
        _NC_CACHE[key] = _build()
    nc = _NC_CACHE[key]

    t, e = _consts()
    in_maps = [
        {
            "labels": np.ascontiguousarray(labels[b]),
            "inputs": np.ascontiguousarray(inputs[b]),
            "tmat": t,
            "evec": e,
        }
        for b in range(B)
    ]
    res = run_bass_kernel_spmd(nc, in_maps, core_ids=list(range(B)))
    return np.stack([res.results[b]["out"] for b in range(B)]).astype(np.float32)


if __name__ == "__main__":
    rng = np.random.default_rng(0)
    labs = rng.random((B, K, H, W), dtype=np.float32)
    ins = rng.standard_normal((B, C, H, W)).astype(np.float32)
    out = kernel(labs, ins)
    print(out.shape, out.dtype, float(np.nanmin(out)), float(np.nanmax(out)))
